# revision 30
# baseline (speedup 1.0000x reference)
"""Distributed Trainium2 kernel for nn_Attention_2654289789382 (sparse_attention).

Math (reference):
    sigma = sigmoid(x @ W_sigma + b_sigma)           (b, h, n)
    den_i = exp(sigma)+1 ;  r_i = 1/den_i = sigmoid(-sigma)   in (0.2689, 0.5)
    prior[i,j] = softmax_j(-|i-j| * r_i)
    out = (prior @ v) reshaped @ W_out + b_out,  v = x @ W_v

Structure exploited:
  * r_i >= 0.2689  =>  prior decays at least as exp(-0.2689 |i-j|): terms with
    |i-j| > 64 are < 4e-8 relative -> banded attention, band half-width 64.
    Per 128-row i-block only 2 j-tiles of 128 (at +-64) contribute.
  * softmax denominator in closed form (two-sided geometric series):
        den_i = 1 + (2z - z^(i+1) - z^(n-i)) / (1-z),  z = exp(-r_i)
  * Q[j,i] = exp(|i-j| * -r_i) built directly in matmul-rhs layout: -r
    partition-broadcast on GpSimd (bounced through DRAM), one bf16 DVE
    multiply against an exact-in-bf16 |dist| master, one ScalarE Exp.
  * AV matmuls bf16, two heads sharing one [128,128] psum (partition ranges
    0:64 / 64:128); 1/den normalization fused into the psum->SBUF move (one
    full-width DVE multiply, bf16 out). out^T lands exactly in the lhsT
    layout the W_out projection needs.

Sharding: 8 cores = 4 batches x 2 sequence halves; no collectives.
"""

import numpy as np
import ml_dtypes

import concourse.bass as bass
import concourse.mybir as mybir
import concourse.tile as tile
from concourse import bacc
from concourse.bass_utils import run_bass_kernel_spmd

F32 = mybir.dt.float32
BF16 = mybir.dt.bfloat16

B, N, D = 4, 2048, 512
H, DH = 8, 64
HALF = N // 2            # 1024 rows per core
PAD = 128                # zero-pad rows at each end of the j range
NJROWS = HALF + 2 * PAD  # 1280 padded j rows per core
NBLK = HALF // 128       # 8 i-blocks per core
NVT = 9                  # V tiles at odd 64-offsets (rows 64k..64k+128, k odd)
CB = 4                   # i-blocks per ARG/exp chunk
NCH = NBLK // CB         # chunks per head

_nc_cache = None


def _build_nc():
    nc = bacc.Bacc("TRN2", target_bir_lowering=False, debug=False)

    xTb = nc.dram_tensor("xTb", [D, NJROWS], BF16, kind="ExternalInput")
    Wvb = nc.dram_tensor("Wvb", [D, D], BF16, kind="ExternalInput")
    Wsb = nc.dram_tensor("Wsb", [D, H], BF16, kind="ExternalInput")
    Wob = nc.dram_tensor("Wob", [D, D], BF16, kind="ExternalInput")
    bsigb = nc.dram_tensor("bsigb", [128, H], F32, kind="ExternalInput")
    bout = nc.dram_tensor("bout", [128, D], F32, kind="ExternalInput")
    # |dist| master: CB copies of [128, 256] (o'=0: dist=|q-p+64|, o'=1: |q-p-64|)
    m2r = nc.dram_tensor("m2r", [128, 256], BF16, kind="ExternalInput")
    ivp1 = nc.dram_tensor("ivp1", [128, NBLK * H], F32, kind="ExternalInput")
    ivnm = nc.dram_tensor("ivnm", [128, NBLK * H], F32, kind="ExternalInput")
    identb = nc.dram_tensor("identb", [128, 128], BF16, kind="ExternalInput")
    out = nc.dram_tensor("out", [HALF, D], F32, kind="ExternalOutput")
    negr_d = nc.dram_tensor("negr_d", [H, HALF], BF16)
    inv_d = nc.dram_tensor("inv_d", [H, HALF], BF16)

    EXP = mybir.ActivationFunctionType.Exp
    SIGM = mybir.ActivationFunctionType.Sigmoid
    MUL = mybir.AluOpType.mult
    ADD = mybir.AluOpType.add

    with tile.TileContext(nc) as tc:
        with (
            tc.tile_pool(name="const", bufs=1) as cpool,
            tc.tile_pool(name="vpool", bufs=1) as vpool,
            tc.tile_pool(name="otpool", bufs=1) as otpool,
            tc.tile_pool(name="sg", bufs=1) as sgpool,
            tc.tile_pool(name="bc", bufs=1) as bcpool,
        ):
            # ---------------- loads ----------------
            xTb_t = []
            for dt in range(4):
                t = cpool.tile([128, NJROWS], BF16, tag=f"xTb{dt}")
                eng = nc.sync if dt % 2 == 0 else nc.scalar
                eng.dma_start(t[:], xTb[dt * 128:(dt + 1) * 128, :])
                xTb_t.append(t)
            Wsb_t, Wvb_t, Wob_t = [], [], []
            for dt in range(4):
                t = cpool.tile([128, H], BF16, tag=f"Wsb{dt}")
                nc.sync.dma_start(t[:], Wsb[dt * 128:(dt + 1) * 128, :])
                Wsb_t.append(t)
            bsig_t = cpool.tile([128, H], F32, tag="bsigb")
            nc.sync.dma_start(bsig_t[:], bsigb[:, :])
            ivp1_t = cpool.tile([128, NBLK * H], F32, tag="ivp1")
            nc.sync.dma_start(ivp1_t[:], ivp1[:, :])
            ivnm_t = cpool.tile([128, NBLK * H], F32, tag="ivnm")
            nc.sync.dma_start(ivnm_t[:], ivnm[:, :])
            identb_t = cpool.tile([128, 128], BF16, tag="identb")
            nc.sync.dma_start(identb_t[:], identb[:, :])
            m2r_t = cpool.tile([128, 256], BF16, tag="m2r")
            nc.scalar.dma_start(m2r_t[:], m2r[:, :])
            for dt in range(4):
                t = cpool.tile([128, D], BF16, tag=f"Wvb{dt}")
                eng = nc.sync if dt % 2 == 0 else nc.scalar
                eng.dma_start(t[:], Wvb[dt * 128:(dt + 1) * 128, :])
                Wvb_t.append(t)
            for dt in range(4):
                t = cpool.tile([128, D], BF16, tag=f"Wob{dt}")
                nc.sync.dma_start(t[:], Wob[dt * 128:(dt + 1) * 128, :])
                Wob_t.append(t)
            bout_t = cpool.tile([128, D], F32, tag="bout")
            nc.sync.dma_start(bout_t[:], bout[:, :])

            # ------------- sigma / closed-form 1/den (first: feeds bcasts) --
            with tc.tile_pool(name="pss", bufs=2, space="PSUM") as pss:
                s_all = sgpool.tile([128, NBLK * H], F32, tag="s_all")
                for b in range(NBLK):
                    ps = pss.tile([128, H], F32, tag="ps")
                    for dt in range(4):
                        nc.tensor.matmul(
                            ps[:],
                            lhsT=xTb_t[dt][:, PAD + b * 128:PAD + (b + 1) * 128],
                            rhs=Wsb_t[dt][:],
                            start=(dt == 0),
                            stop=(dt == 3),
                        )
                    nc.vector.tensor_add(
                        s_all[:, b * H:(b + 1) * H], ps[:], bsig_t[:]
                    )

                sig = sgpool.tile([128, NBLK * H], F32, tag="sig")
                nc.scalar.activation(sig[:], s_all[:], SIGM)
                r_all = sgpool.tile([128, NBLK * H], F32, tag="r_all")
                nc.scalar.activation(r_all[:], sig[:], SIGM, scale=-1.0)
                negr = sgpool.tile([128, NBLK * H], F32, tag="negr")
                nc.vector.tensor_scalar_mul(negr[:], r_all[:], -1.0)

                # ---- stage -r for broadcast ASAP (R gates the main loop) ----
                negr_b = sgpool.tile([128, NBLK * H], BF16, tag="negr_b")
                nc.vector.tensor_copy(
                    negr_b[:].rearrange("p (h b) -> p h b", b=NBLK),
                    negr[:].rearrange("p (b h) -> p h b", h=H),
                )
                ptn = pss.tile([64, 128], BF16, tag="ptn")
                nc.tensor.transpose(ptn[:], negr_b[:], identb_t[:])
                negrT = sgpool.tile([64, 128], BF16, tag="negrT")
                nc.scalar.copy(negrT[:], ptn[:])
                nc.sync.dma_start(
                    negr_d.ap().rearrange("h (b p) -> (h b) p", p=128),
                    negrT[:, :],
                )
                R_all = bcpool.tile([128, H * HALF], BF16, tag="R_all")
                for h in range(H):
                    eng = nc.sync if h % 2 == 0 else nc.scalar
                    eng.dma_start(
                        R_all[:, h * HALF:(h + 1) * HALF],
                        negr_d[h:h + 1, :].to_broadcast((128, HALF)),
                    )

                # ---- 1/den = w / (w + 2z - A - B),  w = 1 - z ----
                z = sgpool.tile([128, NBLK * H], F32, tag="z")
                nc.scalar.activation(z[:], negr[:], EXP)
                argA = sgpool.tile([128, NBLK * H], F32, tag="argA")
                nc.vector.tensor_mul(argA[:], negr[:], ivp1_t[:])
                expA = sgpool.tile([128, NBLK * H], F32, tag="expA")
                nc.scalar.activation(expA[:], argA[:], EXP)
                argB = sgpool.tile([128, NBLK * H], F32, tag="argB")
                nc.vector.tensor_mul(argB[:], negr[:], ivnm_t[:])
                expB = sgpool.tile([128, NBLK * H], F32, tag="expB")
                nc.scalar.activation(expB[:], argB[:], EXP)
                w = sgpool.tile([128, NBLK * H], F32, tag="w")
                nc.vector.tensor_scalar(w[:], z[:], -1.0, 1.0, MUL, ADD)
                t1 = sgpool.tile([128, NBLK * H], F32, tag="t1")
                nc.vector.tensor_scalar_mul(t1[:], z[:], 2.0)
                nc.vector.tensor_sub(t1[:], t1[:], expA[:])
                nc.vector.tensor_sub(t1[:], t1[:], expB[:])
                u = sgpool.tile([128, NBLK * H], F32, tag="u")
                nc.vector.tensor_add(u[:], w[:], t1[:])
                ru = sgpool.tile([128, NBLK * H], F32, tag="ru")
                nc.vector.reciprocal(ru[:], u[:])
                inv_c = sgpool.tile([128, NBLK * H], F32, tag="inv_c")
                nc.vector.tensor_mul(inv_c[:], w[:], ru[:])

                inv_b = sgpool.tile([128, NBLK * H], BF16, tag="inv_b")
                nc.vector.tensor_copy(
                    inv_b[:].rearrange("p (h b) -> p h b", b=NBLK),
                    inv_c[:].rearrange("p (b h) -> p h b", h=H),
                )
                pti = pss.tile([64, 128], BF16, tag="pti")
                nc.tensor.transpose(pti[:], inv_b[:], identb_t[:])
                invT = sgpool.tile([64, 128], BF16, tag="invT")
                nc.scalar.copy(invT[:], pti[:])

            nc.sync.dma_start(
                inv_d.ap().rearrange("h (b p) -> (h b) p", p=128), invT[:, :]
            )
            # Iv_pair[p, hp*HALF + i] = 1/den[2*hp + (p>=64), i]; one DMA per
            # (head pair, partition half) so hp=0 unblocks early.
            Iv_pair = bcpool.tile([128, 4 * HALF], BF16, tag="Iv_pair")
            for hp in range(4):
                nc.sync.dma_start(
                    Iv_pair[0:64, hp * HALF:(hp + 1) * HALF],
                    inv_d[2 * hp:2 * hp + 1, :].to_broadcast((64, HALF)),
                )
                nc.sync.dma_start(
                    Iv_pair[64:128, hp * HALF:(hp + 1) * HALF],
                    inv_d[2 * hp + 1:2 * hp + 2, :].to_broadcast((64, HALF)),
                )

            # ---------------- V = x @ W_v (9 tiles at odd 64-offsets) -------
            V_t = []
            with tc.tile_pool(name="psv", bufs=2, space="PSUM") as psv:
                for vt_i in range(NVT):
                    k = 2 * vt_i + 1
                    pv = psv.tile([128, D], F32, tag="pv")
                    for dt in range(4):
                        nc.tensor.matmul(
                            pv[:],
                            lhsT=xTb_t[dt][:, 64 * k:64 * k + 128],
                            rhs=Wvb_t[dt][:],
                            start=(dt == 0),
                            stop=(dt == 3),
                        )
                    vt = vpool.tile([128, D], BF16, tag=f"V{vt_i}")
                    nc.scalar.copy(vt[:], pv[:])
                    V_t.append(vt)

            # persistent out^T tiles (bf16): tile hp = heads 2hp, 2hp+1
            outT_t = []
            for t in range(4):
                oT = otpool.tile([128, HALF], BF16, tag=f"oT{t}")
                outT_t.append(oT)

            # ---------------- main loop (head pairs) ----------------
            with (
                tc.tile_pool(name="ap", bufs=5) as apool,
                tc.tile_pool(name="qp", bufs=5) as qpool,
                tc.tile_pool(name="fin", bufs=3) as fpool,
                tc.tile_pool(name="psa", bufs=6, space="PSUM") as psa,
                tc.tile_pool(name="psf", bufs=2, space="PSUM") as psf,
            ):
                for ch in range(NCH):
                    for hp in range(4):
                        Qs = []
                        for hh in range(2):
                            h = 2 * hp + hh
                            R = R_all[:, h * HALF + ch * CB * 128:
                                      h * HALF + (ch + 1) * CB * 128]
                            ARG = apool.tile([128, CB * 256], BF16, tag="ARG")
                            nc.vector.tensor_tensor(
                                ARG[:].rearrange("p (b o q) -> p b o q", b=CB, o=2),
                                m2r_t[:]
                                .rearrange("p (one o q) -> p one o q", one=1, o=2)
                                .broadcast_to((128, CB, 2, 128)),
                                R.rearrange("p (b one q) -> p b one q", b=CB, one=1)
                                .broadcast_to((128, CB, 2, 128)),
                                op=MUL,
                            )
                            Q = qpool.tile([128, CB * 256], BF16, tag="Q")
                            nc.scalar.activation(Q[:], ARG[:], EXP)
                            Qs.append(Q)
                        for bi in range(CB):
                            b = ch * CB + bi
                            # full 128-col V-pair as lhsT -> FWL weight loads;
                            # each head keeps its own psum tile and only its
                            # partition half (even: 0:64, odd: 64:128) is read
                            for hh in range(2):
                                pav = psa.tile([128, 128], F32, tag="pav")
                                for o in range(2):
                                    nc.tensor.matmul(
                                        pav[:],
                                        lhsT=V_t[b + o][:, hp * 128:(hp + 1) * 128],
                                        rhs=Qs[hh][:, bi * 256 + o * 128:
                                                   bi * 256 + (o + 1) * 128],
                                        start=(o == 0),
                                        stop=(o == 1),
                                    )
                                rows = slice(hh * 64, (hh + 1) * 64)
                                nc.vector.tensor_mul(
                                    outT_t[hp][rows, b * 128:(b + 1) * 128],
                                    pav[rows, :],
                                    Iv_pair[rows, hp * HALF + b * 128:
                                            hp * HALF + (b + 1) * 128],
                                )
                    # ---- projection for this chunk's blocks ----
                    for bi in range(CB):
                        b = ch * CB + bi
                        cols = slice(b * 128, (b + 1) * 128)
                        pf = psf.tile([128, D], F32, tag="pf")
                        for t in range(4):
                            nc.tensor.matmul(
                                pf[:],
                                lhsT=outT_t[t][:, cols],
                                rhs=Wob_t[t][:],
                                start=(t == 0),
                                stop=(t == 3),
                            )
                        fin = fpool.tile([128, D], F32, tag="fin")
                        nc.vector.tensor_add(fin[:], pf[:], bout_t[:])
                        eng = nc.sync if b % 2 == 0 else nc.scalar
                        eng.dma_start(out[cols, :], fin[:])

    nc.compile()
    return nc


def _make_in_maps(x, W_v, W_sigma, b_sigma, W_out, b_out):
    bf = ml_dtypes.bfloat16
    m2r1 = np.empty((128, 256), dtype=np.float32)
    p = np.arange(128, dtype=np.float32)[:, None]
    q = np.arange(128, dtype=np.float32)[None, :]
    for o in range(2):
        m2r1[:, o * 128:(o + 1) * 128] = np.abs(q - p + 64.0 - 128.0 * o)
    m2r = m2r1.astype(bf)
    identb = np.eye(128, dtype=np.float32).astype(bf)

    Wvb = np.ascontiguousarray(W_v.astype(bf))
    Wsb = np.ascontiguousarray(W_sigma.astype(bf))
    Wob = np.ascontiguousarray(W_out.astype(bf))
    bsig_b = np.broadcast_to(b_sigma[None, :], (128, H)).copy().astype(np.float32)
    bout_b = np.broadcast_to(b_out[None, :], (128, D)).copy().astype(np.float32)

    in_maps = []
    for c in range(8):
        bb, half = c // 2, c % 2
        i_start = half * HALF
        xp = np.zeros((NJROWS, D), dtype=np.float32)
        j_lo = max(0, i_start - PAD)
        j_hi = min(N, i_start + HALF + PAD)
        xp[j_lo - (i_start - PAD):j_hi - (i_start - PAD)] = x[bb, j_lo:j_hi]
        xTb = np.ascontiguousarray(xp.T.astype(bf))

        pcol = np.arange(128, dtype=np.float32)[:, None]
        blk = np.arange(NBLK, dtype=np.float32)[None, :]
        i_abs = i_start + blk * 128 + pcol                     # [128, NBLK]
        ivp1 = np.repeat(i_abs + 1.0, H, axis=1).astype(np.float32)
        ivnm = np.repeat(float(N) - i_abs, H, axis=1).astype(np.float32)

        in_maps.append(
            {
                "xTb": xTb,
                "Wvb": Wvb,
                "Wsb": Wsb,
                "Wob": Wob,
                "bsigb": bsig_b,
                "bout": bout_b,
                "m2r": m2r,
                "ivp1": ivp1,
                "ivnm": ivnm,
                "identb": identb,
            }
        )
    return in_maps


def kernel(x, W_v, W_sigma, b_sigma, W_out, b_out):
    global _nc_cache
    x = np.asarray(x, dtype=np.float32)
    W_v = np.asarray(W_v, dtype=np.float32)
    W_sigma = np.asarray(W_sigma, dtype=np.float32)
    b_sigma = np.asarray(b_sigma, dtype=np.float32)
    W_out = np.asarray(W_out, dtype=np.float32)
    b_out = np.asarray(b_out, dtype=np.float32)

    if _nc_cache is None:
        _nc_cache = _build_nc()
    nc = _nc_cache

    in_maps = _make_in_maps(x, W_v, W_sigma, b_sigma, W_out, b_out)
    res = run_bass_kernel_spmd(nc, in_maps, core_ids=list(range(8)))

    out = np.empty((B, N, D), dtype=np.float32)
    for c in range(8):
        bb, half = c // 2, c % 2
        out[bb, half * HALF:(half + 1) * HALF, :] = res.results[c]["out"]
    return out


# revision 31
# speedup vs baseline: 1.1778x; 1.1778x over previous
"""Distributed Trainium2 kernel for nn_Attention_2654289789382 (sparse_attention).

Math (reference):
    sigma = sigmoid(x @ W_sigma + b_sigma)           (b, h, n)
    den_i = exp(sigma)+1 ;  r_i = 1/den_i = sigmoid(-sigma)   in (0.2689, 0.5)
    prior[i,j] = softmax_j(-|i-j| * r_i)
    out = (prior @ v) reshaped @ W_out + b_out,  v = x @ W_v

Structure exploited:
  * r_i >= 0.2689  =>  prior decays at least as exp(-0.2689 |i-j|): terms with
    |i-j| > 64 are < 4e-8 relative -> banded attention, band half-width 64.
    Per 128-row i-block only 2 j-tiles of 128 (at +-64) contribute.
  * softmax denominator in closed form (two-sided geometric series):
        den_i = 1 + (2z - z^(i+1) - z^(n-i)) / (1-z),  z = exp(-r_i)
  * Q[j,i] = exp(|i-j| * -r_i) built directly in matmul-rhs layout: -r
    partition-broadcast on GpSimd (bounced through DRAM), one bf16 DVE
    multiply against an exact-in-bf16 |dist| master, one ScalarE Exp.
  * AV matmuls bf16, two heads sharing one [128,128] psum (partition ranges
    0:64 / 64:128); 1/den normalization fused into the psum->SBUF move (one
    full-width DVE multiply, bf16 out). out^T lands exactly in the lhsT
    layout the W_out projection needs.

Sharding: 8 cores = 4 batches x 2 sequence halves; no collectives.
"""

import numpy as np
import ml_dtypes

import concourse.bass as bass
import concourse.mybir as mybir
import concourse.tile as tile
from concourse import bacc
from concourse.bass_utils import run_bass_kernel_spmd

F32 = mybir.dt.float32
BF16 = mybir.dt.bfloat16

B, N, D = 4, 2048, 512
H, DH = 8, 64
HALF = N // 2            # 1024 rows per core
PAD = 128                # zero-pad rows at each end of the j range
NJROWS = HALF + 2 * PAD  # 1280 padded j rows per core
NBLK = HALF // 128       # 8 i-blocks per core
NVT = 9                  # V tiles at odd 64-offsets (rows 64k..64k+128, k odd)
CB = 4                   # i-blocks per ARG/exp chunk
NCH = NBLK // CB         # chunks per head

_nc_cache = None


def _build_nc():
    nc = bacc.Bacc("TRN2", target_bir_lowering=False, debug=False)

    xTb = nc.dram_tensor("xTb", [D, NJROWS], BF16, kind="ExternalInput")
    Wvb = nc.dram_tensor("Wvb", [D, D], BF16, kind="ExternalInput")
    Wsb = nc.dram_tensor("Wsb", [D, H], BF16, kind="ExternalInput")
    Wob = nc.dram_tensor("Wob", [D, D], BF16, kind="ExternalInput")
    bsigb = nc.dram_tensor("bsigb", [128, H], F32, kind="ExternalInput")
    bout = nc.dram_tensor("bout", [128, D], F32, kind="ExternalInput")
    # |dist| master: CB copies of [128, 256] (o'=0: dist=|q-p+64|, o'=1: |q-p-64|)
    m2r = nc.dram_tensor("m2r", [128, 256], BF16, kind="ExternalInput")
    ivp1 = nc.dram_tensor("ivp1", [128, NBLK * H], F32, kind="ExternalInput")
    ivnm = nc.dram_tensor("ivnm", [128, NBLK * H], F32, kind="ExternalInput")
    identb = nc.dram_tensor("identb", [128, 128], BF16, kind="ExternalInput")
    out = nc.dram_tensor("out", [HALF, D], F32, kind="ExternalOutput")
    negr_d = nc.dram_tensor("negr_d", [H, HALF], BF16)
    inv_d = nc.dram_tensor("inv_d", [H, HALF], BF16)

    EXP = mybir.ActivationFunctionType.Exp
    SIGM = mybir.ActivationFunctionType.Sigmoid
    MUL = mybir.AluOpType.mult
    ADD = mybir.AluOpType.add

    with tile.TileContext(nc) as tc:
        with (
            tc.tile_pool(name="const", bufs=1) as cpool,
            tc.tile_pool(name="vpool", bufs=1) as vpool,
            tc.tile_pool(name="otpool", bufs=1) as otpool,
            tc.tile_pool(name="sg", bufs=1) as sgpool,
            tc.tile_pool(name="bc", bufs=1) as bcpool,
        ):
            # ---------------- loads ----------------
            xTb_t = []
            for dt in range(4):
                t = cpool.tile([128, NJROWS], BF16, tag=f"xTb{dt}")
                eng = nc.sync if dt % 2 == 0 else nc.scalar
                eng.dma_start(t[:], xTb[dt * 128:(dt + 1) * 128, :])
                xTb_t.append(t)
            Wsb_t, Wvb_t, Wob_t = [], [], []
            for dt in range(4):
                t = cpool.tile([128, H], BF16, tag=f"Wsb{dt}")
                nc.sync.dma_start(t[:], Wsb[dt * 128:(dt + 1) * 128, :])
                Wsb_t.append(t)
            bsig_t = cpool.tile([128, H], F32, tag="bsigb")
            nc.sync.dma_start(bsig_t[:], bsigb[:, :])
            ivp1_t = cpool.tile([128, NBLK * H], F32, tag="ivp1")
            nc.sync.dma_start(ivp1_t[:], ivp1[:, :])
            ivnm_t = cpool.tile([128, NBLK * H], F32, tag="ivnm")
            nc.sync.dma_start(ivnm_t[:], ivnm[:, :])
            identb_t = cpool.tile([128, 128], BF16, tag="identb")
            nc.sync.dma_start(identb_t[:], identb[:, :])
            m2r_t = cpool.tile([128, 256], BF16, tag="m2r")
            nc.scalar.dma_start(m2r_t[:], m2r[:, :])
            for dt in range(4):
                t = cpool.tile([128, D], BF16, tag=f"Wvb{dt}")
                eng = nc.sync if dt % 2 == 0 else nc.scalar
                eng.dma_start(t[:], Wvb[dt * 128:(dt + 1) * 128, :])
                Wvb_t.append(t)
            for dt in range(4):
                t = cpool.tile([128, D], BF16, tag=f"Wob{dt}")
                nc.sync.dma_start(t[:], Wob[dt * 128:(dt + 1) * 128, :])
                Wob_t.append(t)
            bout_t = cpool.tile([128, D], F32, tag="bout")
            nc.sync.dma_start(bout_t[:], bout[:, :])

            # ------------- sigma / closed-form 1/den (first: feeds bcasts) --
            with tc.tile_pool(name="pss", bufs=2, space="PSUM") as pss:
                s_all = sgpool.tile([128, NBLK * H], F32, tag="s_all")
                for b in range(NBLK):
                    ps = pss.tile([128, H], F32, tag="ps")
                    for dt in range(4):
                        nc.tensor.matmul(
                            ps[:],
                            lhsT=xTb_t[dt][:, PAD + b * 128:PAD + (b + 1) * 128],
                            rhs=Wsb_t[dt][:],
                            start=(dt == 0),
                            stop=(dt == 3),
                        )
                    nc.vector.tensor_add(
                        s_all[:, b * H:(b + 1) * H], ps[:], bsig_t[:]
                    )

                sig = sgpool.tile([128, NBLK * H], F32, tag="sig")
                nc.scalar.activation(sig[:], s_all[:], SIGM)
                r_all = sgpool.tile([128, NBLK * H], F32, tag="r_all")
                nc.scalar.activation(r_all[:], sig[:], SIGM, scale=-1.0)
                negr = sgpool.tile([128, NBLK * H], F32, tag="negr")
                nc.vector.tensor_scalar_mul(negr[:], r_all[:], -1.0)

                # ---- stage -r for broadcast ASAP (R gates the main loop) ----
                negr_b = sgpool.tile([128, NBLK * H], BF16, tag="negr_b")
                nc.vector.tensor_copy(
                    negr_b[:].rearrange("p (h b) -> p h b", b=NBLK),
                    negr[:].rearrange("p (b h) -> p h b", h=H),
                )
                ptn = pss.tile([64, 128], BF16, tag="ptn")
                nc.tensor.transpose(ptn[:], negr_b[:], identb_t[:])
                negrT = sgpool.tile([64, 128], BF16, tag="negrT")
                nc.scalar.copy(negrT[:], ptn[:])
                nc.sync.dma_start(
                    negr_d.ap().rearrange("h (b p) -> (h b) p", p=128),
                    negrT[:, :],
                )
                R_all = bcpool.tile([128, H * HALF], BF16, tag="R_all")
                for h in range(H):
                    eng = nc.sync if h % 2 == 0 else nc.scalar
                    eng.dma_start(
                        R_all[:, h * HALF:(h + 1) * HALF],
                        negr_d[h:h + 1, :].to_broadcast((128, HALF)),
                    )

                # ---- 1/den = w / (w + 2z - A - B),  w = 1 - z ----
                z = sgpool.tile([128, NBLK * H], F32, tag="z")
                nc.scalar.activation(z[:], negr[:], EXP)
                argA = sgpool.tile([128, NBLK * H], F32, tag="argA")
                nc.vector.tensor_mul(argA[:], negr[:], ivp1_t[:])
                expA = sgpool.tile([128, NBLK * H], F32, tag="expA")
                nc.scalar.activation(expA[:], argA[:], EXP)
                argB = sgpool.tile([128, NBLK * H], F32, tag="argB")
                nc.vector.tensor_mul(argB[:], negr[:], ivnm_t[:])
                expB = sgpool.tile([128, NBLK * H], F32, tag="expB")
                nc.scalar.activation(expB[:], argB[:], EXP)
                w = sgpool.tile([128, NBLK * H], F32, tag="w")
                nc.vector.tensor_scalar(w[:], z[:], -1.0, 1.0, MUL, ADD)
                t1 = sgpool.tile([128, NBLK * H], F32, tag="t1")
                nc.vector.tensor_scalar_mul(t1[:], z[:], 2.0)
                nc.vector.tensor_sub(t1[:], t1[:], expA[:])
                nc.vector.tensor_sub(t1[:], t1[:], expB[:])
                u = sgpool.tile([128, NBLK * H], F32, tag="u")
                nc.vector.tensor_add(u[:], w[:], t1[:])
                ru = sgpool.tile([128, NBLK * H], F32, tag="ru")
                nc.vector.reciprocal(ru[:], u[:])
                inv_c = sgpool.tile([128, NBLK * H], F32, tag="inv_c")
                nc.vector.tensor_mul(inv_c[:], w[:], ru[:])

                inv_b = sgpool.tile([128, NBLK * H], BF16, tag="inv_b")
                nc.vector.tensor_copy(
                    inv_b[:].rearrange("p (h b) -> p h b", b=NBLK),
                    inv_c[:].rearrange("p (b h) -> p h b", h=H),
                )
                pti = pss.tile([64, 128], BF16, tag="pti")
                nc.tensor.transpose(pti[:], inv_b[:], identb_t[:])
                invT = sgpool.tile([64, 128], BF16, tag="invT")
                nc.scalar.copy(invT[:], pti[:])

            nc.sync.dma_start(
                inv_d.ap().rearrange("h (b p) -> (h b) p", p=128), invT[:, :]
            )
            # Iv_pair[p, hp*HALF + i] = 1/den[2*hp + (p>=64), i]; one DMA per
            # (head pair, partition half) so hp=0 unblocks early.
            Iv_pair = bcpool.tile([128, 4 * HALF], BF16, tag="Iv_pair")
            for hp in range(4):
                nc.sync.dma_start(
                    Iv_pair[0:64, hp * HALF:(hp + 1) * HALF],
                    inv_d[2 * hp:2 * hp + 1, :].to_broadcast((64, HALF)),
                )
                nc.sync.dma_start(
                    Iv_pair[64:128, hp * HALF:(hp + 1) * HALF],
                    inv_d[2 * hp + 1:2 * hp + 2, :].to_broadcast((64, HALF)),
                )

            # ---------------- V = x @ W_v (9 tiles at odd 64-offsets) -------
            V_t = []
            with tc.tile_pool(name="psv", bufs=2, space="PSUM") as psv:
                for vt_i in range(NVT):
                    k = 2 * vt_i + 1
                    pv = psv.tile([128, D], F32, tag="pv")
                    for dt in range(4):
                        nc.tensor.matmul(
                            pv[:],
                            lhsT=xTb_t[dt][:, 64 * k:64 * k + 128],
                            rhs=Wvb_t[dt][:],
                            start=(dt == 0),
                            stop=(dt == 3),
                        )
                    vt = vpool.tile([128, D], BF16, tag=f"V{vt_i}")
                    nc.scalar.copy(vt[:], pv[:])
                    V_t.append(vt)

            # persistent out^T tiles (bf16): tile hp = heads 2hp, 2hp+1
            outT_t = []
            for t in range(4):
                oT = otpool.tile([128, HALF], BF16, tag=f"oT{t}")
                outT_t.append(oT)

            # ---------------- main loop (head pairs) ----------------
            with (
                tc.tile_pool(name="ap", bufs=5) as apool,
                tc.tile_pool(name="qp", bufs=5) as qpool,
                tc.tile_pool(name="fin", bufs=3) as fpool,
                tc.tile_pool(name="psa", bufs=5, space="PSUM") as psa,
                tc.tile_pool(name="psf", bufs=2, space="PSUM") as psf,
            ):
                for ch in range(NCH):
                    for hp in range(4):
                        Qs = []
                        for hh in range(2):
                            h = 2 * hp + hh
                            R = R_all[:, h * HALF + ch * CB * 128:
                                      h * HALF + (ch + 1) * CB * 128]
                            ARG = apool.tile([128, CB * 256], BF16, tag="ARG")
                            nc.vector.tensor_tensor(
                                ARG[:].rearrange("p (b o q) -> p b o q", b=CB, o=2),
                                m2r_t[:]
                                .rearrange("p (one o q) -> p one o q", one=1, o=2)
                                .broadcast_to((128, CB, 2, 128)),
                                R.rearrange("p (b one q) -> p b one q", b=CB, one=1)
                                .broadcast_to((128, CB, 2, 128)),
                                op=MUL,
                            )
                            Q = qpool.tile([128, CB * 256], BF16, tag="Q")
                            nc.scalar.activation(Q[:], ARG[:], EXP)
                            Qs.append(Q)
                        for bi in range(CB):
                            b = ch * CB + bi
                            pav = psa.tile([128, 128], F32, tag="pav")
                            for hh in range(2):
                                h = 2 * hp + hh
                                for o in range(2):
                                    nc.tensor.matmul(
                                        pav[hh * 64:(hh + 1) * 64, :],
                                        lhsT=V_t[b + o][:, h * 64:(h + 1) * 64],
                                        rhs=Qs[hh][:, bi * 256 + o * 128:
                                                   bi * 256 + (o + 1) * 128],
                                        start=(o == 0),
                                        stop=(o == 1),
                                    )
                            nc.vector.tensor_mul(
                                outT_t[hp][:, b * 128:(b + 1) * 128],
                                pav[:],
                                Iv_pair[:, hp * HALF + b * 128:
                                        hp * HALF + (b + 1) * 128],
                            )
                    # ---- projection for this chunk's blocks ----
                    for bi in range(CB):
                        b = ch * CB + bi
                        cols = slice(b * 128, (b + 1) * 128)
                        pf = psf.tile([128, D], F32, tag="pf")
                        for t in range(4):
                            nc.tensor.matmul(
                                pf[:],
                                lhsT=outT_t[t][:, cols],
                                rhs=Wob_t[t][:],
                                start=(t == 0),
                                stop=(t == 3),
                            )
                        fin = fpool.tile([128, D], F32, tag="fin")
                        nc.vector.tensor_add(fin[:], pf[:], bout_t[:])
                        eng = nc.sync if b % 2 == 0 else nc.scalar
                        eng.dma_start(out[cols, :], fin[:])

    nc.compile()
    return nc


def _make_in_maps(x, W_v, W_sigma, b_sigma, W_out, b_out):
    bf = ml_dtypes.bfloat16
    m2r1 = np.empty((128, 256), dtype=np.float32)
    p = np.arange(128, dtype=np.float32)[:, None]
    q = np.arange(128, dtype=np.float32)[None, :]
    for o in range(2):
        m2r1[:, o * 128:(o + 1) * 128] = np.abs(q - p + 64.0 - 128.0 * o)
    m2r = m2r1.astype(bf)
    identb = np.eye(128, dtype=np.float32).astype(bf)

    Wvb = np.ascontiguousarray(W_v.astype(bf))
    Wsb = np.ascontiguousarray(W_sigma.astype(bf))
    Wob = np.ascontiguousarray(W_out.astype(bf))
    bsig_b = np.broadcast_to(b_sigma[None, :], (128, H)).copy().astype(np.float32)
    bout_b = np.broadcast_to(b_out[None, :], (128, D)).copy().astype(np.float32)

    in_maps = []
    for c in range(8):
        bb, half = c // 2, c % 2
        i_start = half * HALF
        xp = np.zeros((NJROWS, D), dtype=np.float32)
        j_lo = max(0, i_start - PAD)
        j_hi = min(N, i_start + HALF + PAD)
        xp[j_lo - (i_start - PAD):j_hi - (i_start - PAD)] = x[bb, j_lo:j_hi]
        xTb = np.ascontiguousarray(xp.T.astype(bf))

        pcol = np.arange(128, dtype=np.float32)[:, None]
        blk = np.arange(NBLK, dtype=np.float32)[None, :]
        i_abs = i_start + blk * 128 + pcol                     # [128, NBLK]
        ivp1 = np.repeat(i_abs + 1.0, H, axis=1).astype(np.float32)
        ivnm = np.repeat(float(N) - i_abs, H, axis=1).astype(np.float32)

        in_maps.append(
            {
                "xTb": xTb,
                "Wvb": Wvb,
                "Wsb": Wsb,
                "Wob": Wob,
                "bsigb": bsig_b,
                "bout": bout_b,
                "m2r": m2r,
                "ivp1": ivp1,
                "ivnm": ivnm,
                "identb": identb,
            }
        )
    return in_maps


def kernel(x, W_v, W_sigma, b_sigma, W_out, b_out):
    global _nc_cache
    x = np.asarray(x, dtype=np.float32)
    W_v = np.asarray(W_v, dtype=np.float32)
    W_sigma = np.asarray(W_sigma, dtype=np.float32)
    b_sigma = np.asarray(b_sigma, dtype=np.float32)
    W_out = np.asarray(W_out, dtype=np.float32)
    b_out = np.asarray(b_out, dtype=np.float32)

    if _nc_cache is None:
        _nc_cache = _build_nc()
    nc = _nc_cache

    in_maps = _make_in_maps(x, W_v, W_sigma, b_sigma, W_out, b_out)
    res = run_bass_kernel_spmd(nc, in_maps, core_ids=list(range(8)))

    out = np.empty((B, N, D), dtype=np.float32)
    for c in range(8):
        bb, half = c // 2, c % 2
        out[bb, half * HALF:(half + 1) * HALF, :] = res.results[c]["out"]
    return out
